# revision 66
# baseline (speedup 1.0000x reference)
"""Trainium2 Bass kernel for GQA attention (B=2, S=2048, HID=2048, H=16, HKV=4, RoPE, causal).

Sharding: TP=4 over GQA groups (4 Q heads + 1 KV head per core) x DP=2 over batch.
Core i -> (batch = i // 4, group = i % 4). Each core computes a partial output
x @ Wo_shard for its head group; host sums the 4 partials per batch.

v2 pipeline (all matmul operands fp16, fp32 PSUM accumulation):
  - projections per seq chunk c (xt streamed c-major from DRAM), RoPE on DVE,
    PE-transpose q/k -> qT/kT [d, s]; v stays [s, d].
  - attention per 512-query strip, scores computed TRANSPOSED [sk, sq]
    (kT chunk stationary, qT strip moving) so exp (ACT) writes P^T into SBUF
    directly -- no PE P-transposes, no DVE P copies. Causal triangle of the
    diagonal block is zeroed post-exp by a Pool affine_select.
  - softmax row-sums: fp16 DVE accumulation of P^T chunks, then ONE PE
    ones-matmul per (head, strip) that both sums across partitions and
    broadcasts the result to all 128 partitions (HW's gpsimd
    partition_all_reduce measured ~5us/call -- avoid); normalization is
    folded into the PV PSUM->SBUF eviction multiply (DVE).
  - token-level software pipelining: HW measures independent 512-col matmuls
    at ~162ns but PSUM-accumulating chains at ~270ns (RMW bubble), so
    projection matmul pairs are woven between score chunks, and the previous
    strip's O-projection matmuls between PV chunks, hiding both the
    accumulation bubbles and the ACT exp pacing (~1.07us per 512-col chunk).
  - PSUM: qps x2, trps, shared scores/O-proj/rowsum ring x3, pv/kv shared x2
    = 8 banks; fp16 output (host upcasts while summing the 4 TP partials);
    cross-strip pipelining: the next strip's first two score heads are
    emitted during the current strip's tail to prefill the ACT-bound
    final strip.
"""
import sys
sys.path.insert(0, "/opt/trn_rl_repo")
import math
import numpy as np
import concourse.mybir as mybir
import concourse.tile as tile
from concourse import bacc, bass_isa
from concourse.bass_utils import run_bass_kernel_spmd
from concourse.masks import make_identity

F16 = mybir.dt.float16
F32 = mybir.dt.float32
AF = mybir.ActivationFunctionType
ALU = mybir.AluOpType
RED = bass_isa.ReduceOp

NH = 4          # q heads per core
D = 128         # head dim
MASK_VAL = -1e9
EXP_BIAS = -4.0

DEFAULT_BUFS = dict(qps=2, trps=1, scop=3, pv=2,
                    xt=4, q16=2, k16=2, pt=2, racc=2, rsum=2, rcp=2, attnT=2, osb=2)


def build(S=2048, HID=2048, repeat=1, bufs=None, norm=True, out_eng="sp", startup="spread",
          evict="act", racc_pairs=False, unify=False, bias_scalar=False, op_riffle=False,
          pair_prologue=True, out16=True, xstrip=True, spv=False):
    bz = dict(DEFAULT_BUFS)
    if bufs:
        bz.update(bufs)
    SC = S // 128        # seq chunks
    NT = S // 512        # 512-wide query strips
    HC = HID // 128      # hidden (contraction) chunks
    QW = NH * D          # 512: q width per core
    CB = HC * 128        # xt block cols per seq chunk (2048)
    scale = 1.0 / math.sqrt(D)

    nc = bacc.Bacc(None, target_bir_lowering=False, debug=False)
    with tile.TileContext(nc) as tc:
        with tc.tile_pool(name="dram", bufs=1, space="DRAM") as dram:
            # xt c-major: block c holds all HC hid-chunks of seq chunk c
            xt_d = dram.tile([128, SC * CB], F16, kind="ExternalInput", name="xt", uniquify=False)
            wqkv_d = dram.tile([128, HC * (QW + 256)], F16, kind="ExternalInput", name="wqkv", uniquify=False)
            cos_d = dram.tile([128, SC * QW], F16, kind="ExternalInput", name="cos4", uniquify=False)
            sin_d = dram.tile([128, SC * QW], F16, kind="ExternalInput", name="sin4", uniquify=False)
            wo_d = dram.tile([128, NH * HID], F16, kind="ExternalInput", name="wo", uniquify=False)
            out_d = dram.tile([S, HID], F16 if out16 else F32, kind="ExternalOutput", name="out", uniquify=False)

            with tc.tile_pool(name="keep", bufs=1) as keep:
                qT_sb = keep.tile([128, NH * S], F16)   # [d, h*S + s]
                kT_sb = keep.tile([128, S], F16)        # [d, sk]
                v_sb = keep.tile([128, S], F16)         # [sk%128, chunk*128 + d]
                ident = keep.tile([128, 128], F16)
                make_identity(nc, ident[:])
                ones128 = keep.tile([128, 128], F16)
                nc.gpsimd.memset(ones128[:], 1.0)
                ebias = keep.tile([128, 1], F32)
                nc.gpsimd.memset(ebias[:], EXP_BIAS)
                # transposed causal mask for the diagonal 128x128 block:
                # visible (0) iff key_pos (partition) <= query_pos (free)
                cmaskT = keep.tile([128, 128], F32)
                nc.gpsimd.memset(cmaskT[:], 0.0)
                nc.gpsimd.affine_select(
                    out=cmaskT[:], in_=cmaskT[:], compare_op=ALU.is_ge,
                    fill=MASK_VAL, base=0, pattern=[[1, 128]], channel_multiplier=-1,
                )

                from contextlib import ExitStack
                _rep = ExitStack()
                if repeat > 1:
                    _rep.enter_context(tc.For_i(0, repeat, 1))

                with tc.tile_pool(name="pp", bufs=1) as pp, \
                     tc.tile_pool(name="psp", bufs=2, space="PSUM") as psp:
                    # persistent-ish weights (reloaded per repeat iteration)
                    wqkv_sb = pp.tile([128, HC * (QW + 256)], F16)
                    cos_sb = pp.tile([128, SC * QW], F16)
                    sin_sb = pp.tile([128, SC * QW], F16)
                    wo_sb = pp.tile([128, NH * HID], F16)
                    xt_tiles = {}

                    def fetch_x(c, eng_x=None, eng_cs=None):
                        if c >= SC:
                            return
                        xt_tiles[c] = pp.tile([128, CB], F16, tag="xt", bufs=bz["xt"], name=f"xtb{c}")
                        (eng_x or nc.sync).dma_start(out=xt_tiles[c][:], in_=xt_d[:, c * CB:(c + 1) * CB])
                        (eng_cs or nc.sync).dma_start(out=cos_sb[:, c * QW:(c + 1) * QW],
                                                      in_=cos_d[:, c * QW:(c + 1) * QW])
                        (eng_cs or nc.sync).dma_start(out=sin_sb[:, c * QW:(c + 1) * QW],
                                                      in_=sin_d[:, c * QW:(c + 1) * QW])

                    # Startup DMAs spread across idle engines: xt on ACT,
                    # cos/sin on Pool, so nothing queues behind wqkv on SP.
                    for c in range(min(bz["xt"], SC)):
                        if startup == "spread":
                            fetch_x(c, nc.scalar, nc.scalar)
                        else:
                            fetch_x(c)
                    for hh in range(HC):
                        nc.sync.dma_start(out=wqkv_sb[:, hh * (QW + 256):(hh + 1) * (QW + 256)],
                                          in_=wqkv_d[:, hh * (QW + 256):(hh + 1) * (QW + 256)])
                    nc.sync.dma_start(out=wo_sb[:], in_=wo_d[:])

                    def proj_chunk_a(c, state):
                        """First half of projections for seq chunk c."""
                        xt_sb = xt_tiles.pop(c)
                        q_ps = psp.tile([128, QW], F32, tag="qps", bufs=bz["qps"], name=f"qps{c}")
                        kv_ps = psp.tile([128, 256], F32, tag="pv", bufs=bz["pv"], name=f"kvps{c}")
                        state.update(xt_sb=xt_sb, q_ps=q_ps, kv_ps=kv_ps)
                        for hh in range(HC // 2):
                            xk = xt_sb[:, hh * 128:(hh + 1) * 128]
                            nc.tensor.matmul(q_ps[:], xk, wqkv_sb[:, hh * (QW + 256): hh * (QW + 256) + QW],
                                             start=(hh == 0), stop=False)
                            nc.tensor.matmul(kv_ps[:], xk, wqkv_sb[:, hh * (QW + 256) + QW: (hh + 1) * (QW + 256)],
                                             start=(hh == 0), stop=False)

                    def proj_chunk_b(c, state):
                        """Second half of projections for seq chunk c."""
                        xt_sb, q_ps, kv_ps = state["xt_sb"], state["q_ps"], state["kv_ps"]
                        for hh in range(HC // 2, HC):
                            xk = xt_sb[:, hh * 128:(hh + 1) * 128]
                            nc.tensor.matmul(q_ps[:], xk, wqkv_sb[:, hh * (QW + 256): hh * (QW + 256) + QW],
                                             start=False, stop=(hh == HC - 1))
                            nc.tensor.matmul(kv_ps[:], xk, wqkv_sb[:, hh * (QW + 256) + QW: (hh + 1) * (QW + 256)],
                                             start=False, stop=(hh == HC - 1))
                        # prefetch a later chunk into the slot just vacated
                        fetch_x(c + bz["xt"])

                    def proj_chunk(c):
                        st = {}
                        proj_chunk_a(c, st)
                        proj_chunk_b(c, st)
                        proj_rope(c, st)

                    def proj_rope(c, state):
                        """RoPE + transposes for seq chunk c."""
                        q_ps, kv_ps = state["q_ps"], state["kv_ps"]
                        # --- RoPE on q (4 heads batched) ---
                        q4 = q_ps[:].rearrange("p (h d) -> p h d", h=NH)
                        cos4v = cos_sb[:, c * QW:(c + 1) * QW].rearrange("p (h d) -> p h d", h=NH)
                        sin4v = sin_sb[:, c * QW:(c + 1) * QW].rearrange("p (h d) -> p h d", h=NH)
                        rot = pp.tile([128, QW], F32, tag="rot")
                        rot4 = rot[:].rearrange("p (h d) -> p h d", h=NH)
                        nc.vector.tensor_mul(rot4[:, :, 0:64], q4[:, :, 64:128], sin4v[:, :, 0:64])
                        nc.vector.tensor_mul(rot4[:, :, 64:128], q4[:, :, 0:64], sin4v[:, :, 64:128])
                        qc = pp.tile([128, QW], F32, tag="qc")
                        nc.vector.tensor_mul(qc[:], q_ps[:], cos_sb[:, c * QW:(c + 1) * QW])
                        q16 = pp.tile([128, QW], F16, tag="q16", bufs=bz["q16"])
                        nc.vector.tensor_add(q16[:], qc[:], rot[:])
                        # --- RoPE on k (head 0 slices of cos/sin) ---
                        k1 = kv_ps[:, 0:128]
                        cos1 = cos_sb[:, c * QW: c * QW + 128]
                        sin1 = sin_sb[:, c * QW: c * QW + 128]
                        krot = pp.tile([128, 128], F32, tag="krot")
                        nc.vector.tensor_mul(krot[:, 0:64], k1[:, 64:128], sin1[:, 0:64])
                        nc.vector.tensor_mul(krot[:, 64:128], k1[:, 0:64], sin1[:, 64:128])
                        kc_t = pp.tile([128, 128], F32, tag="kc")
                        nc.vector.tensor_mul(kc_t[:], k1, cos1)
                        k16 = pp.tile([128, 128], F16, tag="k16", bufs=bz["k16"])
                        nc.vector.tensor_add(k16[:], kc_t[:], krot[:])
                        # --- v to persistent [s, d] fp16 ---
                        nc.vector.tensor_copy(v_sb[:, c * 128:(c + 1) * 128], kv_ps[:, 128:256])
                        # --- transpose q heads + k into qT/kT ---
                        tr_ps = psp.tile([128, 640], F16, tag="trps", bufs=bz["trps"])
                        for h in range(NH):
                            nc.tensor.transpose(tr_ps[:, h * 128:(h + 1) * 128], q16[:, h * 128:(h + 1) * 128], ident[:])
                        nc.tensor.transpose(tr_ps[:, 512:640], k16[:], ident[:])
                        qT_view = qT_sb[:].rearrange("p (h s) -> p h s", h=NH)[:, :, c * 128:(c + 1) * 128]
                        nc.vector.tensor_copy(qT_view, tr_ps[:, 0:512].rearrange("p (h s) -> p h s", h=NH))
                        nc.vector.tensor_copy(kT_sb[:, c * 128:(c + 1) * 128], tr_ps[:, 512:640])

                    def oproj_group(t, ci, attnT):
                        c = 4 * t + ci
                        osb = pp.tile([128, HID], F16 if out16 else F32, tag="osb", bufs=bz["osb"], name=f"osb{c}")
                        for n in range(HID // 512):
                            op = psp.tile([128, 512], F32, tag="scop", bufs=bz["scop"], name=f"op{c}_{n}")
                            for h in range(NH):
                                nc.tensor.matmul(op[:], attnT[:, h * 512 + ci * 128: h * 512 + (ci + 1) * 128],
                                                 wo_sb[:, h * HID + n * 512: h * HID + (n + 1) * 512],
                                                 start=(h == 0), stop=(h == NH - 1))
                            if evict == "mix":
                                ev = nc.scalar.copy if n % 2 == 0 else nc.vector.tensor_copy
                            else:
                                ev = nc.scalar.copy if evict == "act" else nc.vector.tensor_copy
                            ev(osb[:, n * 512:(n + 1) * 512], op[:])
                        out_e = {"pool": nc.gpsimd, "sp": nc.sync, "act": nc.scalar}[out_eng]
                        out_e.dma_start(out=out_d[c * 128:(c + 1) * 128, :], in_=osb[:])

                    def proj_pair_tokens(c):
                        """Per-hh (q mm, kv mm) pair tokens + final rope token."""
                        st = {}
                        toks = []

                        def pair(hh):
                            if hh == 0:
                                st["xt_sb"] = xt_tiles.pop(c)
                                st["q_ps"] = psp.tile([128, QW], F32, tag="qps", bufs=bz["qps"], name=f"qps{c}")
                                st["kv_ps"] = psp.tile([128, 256], F32, tag="pv", bufs=bz["pv"], name=f"kvps{c}")
                            xk = st["xt_sb"][:, hh * 128:(hh + 1) * 128]
                            nc.tensor.matmul(st["q_ps"][:], xk, wqkv_sb[:, hh * (QW + 256): hh * (QW + 256) + QW],
                                             start=(hh == 0), stop=(hh == HC - 1))
                            nc.tensor.matmul(st["kv_ps"][:], xk, wqkv_sb[:, hh * (QW + 256) + QW: (hh + 1) * (QW + 256)],
                                             start=(hh == 0), stop=(hh == HC - 1))
                            if hh == HC - 1:
                                fetch_x(c + bz["xt"])

                        for hh in range(HC):
                            toks.append(lambda hh=hh: pair(hh))
                        toks.append(lambda: proj_rope(c, st))
                        return toks

                    def oproj_tokens(t, ci, attnT):
                        """Per-(n,h) matmul tokens; evict folded into h==last."""
                        c = 4 * t + ci
                        st = {}
                        toks = []

                        def mmtok(n, h):
                            if n == 0 and h == 0:
                                st["osb"] = pp.tile([128, HID], F16 if out16 else F32, tag="osb", bufs=bz["osb"], name=f"osb{c}")
                            if h == 0:
                                st["op"] = psp.tile([128, 512], F32, tag="scop", bufs=bz["scop"], name=f"op{c}_{n}")
                            nc.tensor.matmul(st["op"][:], attnT[:, h * 512 + ci * 128: h * 512 + (ci + 1) * 128],
                                             wo_sb[:, h * HID + n * 512: h * HID + (n + 1) * 512],
                                             start=(h == 0), stop=(h == NH - 1))
                            if h == NH - 1:
                                if evict == "mix":
                                    ev = nc.scalar.copy if n % 2 == 0 else nc.vector.tensor_copy
                                else:
                                    ev = nc.scalar.copy if evict == "act" else nc.vector.tensor_copy
                                ev(st["osb"][:, n * 512:(n + 1) * 512], st["op"][:])

                        for n in range(HID // 512):
                            for h in range(NH):
                                toks.append(lambda n=n, h=h: mmtok(n, h))

                        def dmatok():
                            out_e = {"pool": nc.gpsimd, "sp": nc.sync, "act": nc.scalar}[out_eng]
                            out_e.dma_start(out=out_d[c * 128:(c + 1) * 128, :], in_=st["osb"][:])
                        toks.append(dmatok)
                        return toks

                    class Feeder:
                        """Drains filler tokens evenly across primary slots."""
                        def __init__(self, toks, slots):
                            self.toks = toks
                            self.per = len(toks) / max(slots, 1)
                            self.acc = 0.0
                            self.i = 0

                        def step(self):
                            self.acc += self.per
                            while self.i < len(self.toks) and self.i < int(self.acc + 1e-9):
                                self.toks[self.i]()
                                self.i += 1

                        def flush(self):
                            while self.i < len(self.toks):
                                self.toks[self.i]()
                                self.i += 1

                    if pair_prologue:
                        # interleave chunk pairs: 4 independent PSUM chains
                        # (q c0, q c1, kv c0, kv c1) hide accumulation bubbles
                        for c0 in (0, 2):
                            la = proj_pair_tokens(c0)
                            lb = proj_pair_tokens(c0 + 1)
                            for x, y in zip(la, lb):
                                x(); y()
                    else:
                        for c in range(4):
                            proj_chunk(c)

                    def make_strip(t, pending_op_toks):
                        nk = 4 * t + 4
                        nxt = [4 * (t + 1) + i for i in range(4)] if t + 1 < NT else []
                        projtoks = [tok for c in nxt for tok in proj_pair_tokens(c)]
                        if not projtoks:
                            # last strip: ACT-paced scores leave PE idle, so
                            # feed O-proj tokens into score slots as well
                            fproj = Feeder(pending_op_toks, NH * nk + NH * (nk + 1))
                            fop = fproj
                            skip_early = True
                        else:
                            fproj = Feeder(projtoks, NH * nk)
                            fop = Feeder(pending_op_toks, NH * (nk + 1))
                            skip_early = False
                        return dict(t=t, nk=nk, fproj=fproj, fop=fop, raccs=[], pts=[],
                                    attnT=None, skip_early=skip_early)

                    def scores_units(Sd, h, early=False):
                        t, nk, fproj = Sd["t"], Sd["nk"], Sd["fproj"]
                        pts, raccs = Sd["pts"], Sd["raccs"]
                        pt_sb = pp.tile([128, nk * 512], F16, tag="pt", bufs=bz["pt"], name=f"pt{t}_{h}")
                        pts.append(pt_sb)
                        racc = pp.tile([128, 512], F16, tag="racc", bufs=bz["racc"], name=f"racc{t}_{h}")
                        raccs.append(racc)
                        units = []

                        def chunk(kc):
                                off = 128 * max(0, kc - 4 * t)
                                w = 512 - off
                                sc = psp.tile([128, 512], F32, tag="scop", bufs=bz["scop"])
                                nc.tensor.matmul(sc[:, off:512], kT_sb[:, kc * 128:(kc + 1) * 128],
                                                 qT_sb[:, h * S + t * 512 + off: h * S + (t + 1) * 512],
                                                 start=True, stop=True)
                                if not early:
                                    fproj.step()
                                nc.scalar.activation(pt_sb[:, kc * 512 + off:(kc + 1) * 512], sc[:, off:512],
                                                     AF.Exp, scale=scale,
                                                     bias=(EXP_BIAS if bias_scalar else ebias[:]))
                                if kc >= 4 * t:
                                    # zero the causal upper triangle of the diagonal
                                    # 128-col block (post-exp) on Pool
                                    nc.gpsimd.affine_select(
                                        out=pt_sb[:, kc * 512 + off: kc * 512 + off + 128],
                                        in_=pt_sb[:, kc * 512 + off: kc * 512 + off + 128],
                                        compare_op=ALU.is_ge, fill=0.0,
                                        base=0, pattern=[[1, 128]], channel_multiplier=-1,
                                    )
                                if off:
                                    nc.gpsimd.memset(pt_sb[:, kc * 512: kc * 512 + off], 0.0)
                                if kc == 0:
                                    nc.vector.tensor_copy(racc[:], pt_sb[:, 0:512])
                                else:
                                    nc.vector.tensor_add(racc[:, off:512], racc[:, off:512],
                                                         pt_sb[:, kc * 512 + off:(kc + 1) * 512])

                        for kc in range(nk):
                            units.append(lambda kc=kc: chunk(kc))
                        return units

                    def scores_head(Sd, h, early=False):
                        for u in scores_units(Sd, h, early):
                            u()
                    def pv_units(Sd, h):
                        t, nk, fop = Sd["t"], Sd["nk"], Sd["fop"]
                        pts, raccs = Sd["pts"], Sd["raccs"]
                        st = {}

                        def chunk(kc):
                            if kc == 0:
                                if Sd["attnT"] is None:
                                    Sd["attnT"] = pp.tile([128, NH * 512], F16, tag="attnT",
                                                          bufs=bz["attnT"], name=f"attnT{t}")
                                if norm:
                                    # ones-matmul: sums racc across partitions AND
                                    # broadcasts to all 128 output partitions
                                    rsum = psp.tile([128, 512], F32, tag="scop", bufs=bz["scop"], name=f"rs{t}_{h}")
                                    nc.tensor.matmul(rsum[:], ones128[:], raccs[h][:], start=True, stop=True)
                                    st["rcp"] = pp.tile([128, 512], F32, tag="rcp", bufs=bz["rcp"], name=f"rcp{t}_{h}")
                                    nc.vector.reciprocal(st["rcp"][:], rsum[:])
                                fop.step()
                                st["pv"] = psp.tile([128, 512], F32, tag="pv", bufs=bz["pv"], name=f"pvt{t}_{h}")
                            nc.tensor.matmul(st["pv"][:], v_sb[:, kc * 128:(kc + 1) * 128],
                                             pts[h][:, kc * 512:(kc + 1) * 512],
                                             start=(kc == 0), stop=(kc == nk - 1))
                            fop.step()
                            if kc == nk - 1:
                                attnT = Sd["attnT"]
                                if norm:
                                    nc.vector.tensor_mul(attnT[:, h * 512:(h + 1) * 512], st["pv"][:], st["rcp"][:])
                                else:
                                    nc.vector.tensor_copy(attnT[:, h * 512:(h + 1) * 512], st["pv"][:])

                        return [lambda kc=kc: chunk(kc) for kc in range(nk)]

                    def pv_head(Sd, h):
                        for u in pv_units(Sd, h):
                            u()

                    def seg(su, pu):
                        i = j = 0
                        while i < len(su) or j < len(pu):
                            if i < len(su):
                                su[i](); i += 1
                            if j < len(pu):
                                pu[j](); j += 1

                    # cross-strip software pipeline: the next strip's first two
                    # score heads (ACT exp work) are emitted during this
                    # strip's tail, where ACT otherwise idles
                    if xstrip and spv:
                        # head h+1 score chunks interleave with head h PV
                        # chunks so ACT's exp stream never starves between
                        # heads (and PV accumulation bubbles hide behind
                        # independent score matmuls)
                        cur = make_strip(0, [])
                        scores_head(cur, 0)
                        scores_head(cur, 1)
                        for t in range(NT):
                            seg(scores_units(cur, 2), pv_units(cur, 0))
                            seg(scores_units(cur, 3), pv_units(cur, 1))
                            cur["fproj"].flush()
                            cur["fop"].flush()
                            optoks = [tok for ci in range(4)
                                      for tok in oproj_tokens(t, ci, cur["attnT"])]
                            if t + 1 < NT:
                                nxt = make_strip(t + 1, optoks)
                                seg(scores_units(nxt, 0, early=nxt["skip_early"]),
                                    pv_units(cur, 2))
                                seg(scores_units(nxt, 1, early=nxt["skip_early"]),
                                    pv_units(cur, 3))
                                cur = nxt
                            else:
                                pv_head(cur, 2)
                                pv_head(cur, 3)
                                for tok in optoks:
                                    tok()
                    elif xstrip:
                        cur = make_strip(0, [])
                        scores_head(cur, 0)
                        scores_head(cur, 1)
                        for t in range(NT):
                            pv_head(cur, 0)
                            scores_head(cur, 2)
                            pv_head(cur, 1)
                            scores_head(cur, 3)
                            pv_head(cur, 2)
                            cur["fproj"].flush()
                            cur["fop"].flush()
                            optoks = [tok for ci in range(4)
                                      for tok in oproj_tokens(t, ci, cur["attnT"])]
                            if t + 1 < NT:
                                nxt = make_strip(t + 1, optoks)
                                scores_head(nxt, 0, early=nxt["skip_early"])
                                pv_head(cur, 3)
                                scores_head(nxt, 1, early=nxt["skip_early"])
                                cur = nxt
                            else:
                                pv_head(cur, 3)
                                if op_riffle:
                                    # tail drain: riffle ci pairs so adjacent
                                    # O-proj matmuls alternate PSUM banks (the
                                    # scop ring is free of scores here)
                                    n4 = len(optoks) // 4
                                    for b0 in range(0, 4, 2):
                                        la = optoks[b0 * n4:(b0 + 1) * n4]
                                        lb = optoks[(b0 + 1) * n4:(b0 + 2) * n4]
                                        for x, y in zip(la, lb):
                                            x(); y()
                                else:
                                    for tok in optoks:
                                        tok()
                    else:
                        pending = []
                        for t in range(NT):
                            Sd = make_strip(t, pending)
                            scores_head(Sd, 0)
                            scores_head(Sd, 1)
                            pv_head(Sd, 0)
                            scores_head(Sd, 2)
                            pv_head(Sd, 1)
                            scores_head(Sd, 3)
                            pv_head(Sd, 2)
                            pv_head(Sd, 3)
                            Sd["fproj"].flush()
                            Sd["fop"].flush()
                            pending = [tok for ci in range(4)
                                       for tok in oproj_tokens(t, ci, Sd["attnT"])]
                        for tok in pending:
                            tok()
                _rep.close()
    nc.compile()
    return nc


def _chunk_major(a, rows=128):
    """[R, C] -> [128, (R//128)*C] with row-chunk-major free layout."""
    r, c = a.shape
    return np.ascontiguousarray(a.reshape(r // rows, rows, c).transpose(1, 0, 2).reshape(rows, (r // rows) * c))


def make_in_map(x_b, cos, sin, wq_g, wk_g, wv_g, wo_g, S, HID):
    SC = S // 128
    HC = HID // 128
    # xt c-major: xt[p, c*HC*128 + hh*128 + f] = x_b.T[hh*128+p, c*128+f]
    xT = np.ascontiguousarray(x_b.T).astype(np.float16)  # [HID, S]
    xt = xT.reshape(HC, 128, SC, 128).transpose(1, 2, 0, 3).reshape(128, SC * HC * 128)
    wqkv = _chunk_major(np.concatenate([wq_g, wk_g, wv_g], axis=1)).astype(np.float16)
    cosr = cos[:S].reshape(SC, 128, D)
    cos4 = np.repeat(cosr[:, :, None, :], NH, axis=2).transpose(1, 0, 2, 3).reshape(128, SC * NH * D)
    sing = np.concatenate([-sin[:S, :64], sin[:S, 64:]], axis=1).reshape(SC, 128, D)
    sin4 = np.repeat(sing[:, :, None, :], NH, axis=2).transpose(1, 0, 2, 3).reshape(128, SC * NH * D)
    wo = _chunk_major(wo_g).astype(np.float16)
    return {
        "xt": np.ascontiguousarray(xt),
        "wqkv": wqkv,
        "cos4": np.ascontiguousarray(cos4).astype(np.float16),
        "sin4": np.ascontiguousarray(sin4).astype(np.float16),
        "wo": wo,
    }


_NC_CACHE = {}

# best-measured configuration (updated as experiments conclude)
BEST_BUFS = dict(DEFAULT_BUFS)


def _get_nc(S, HID):
    key = (S, HID)
    if key not in _NC_CACHE:
        _NC_CACHE[key] = build(S, HID, bufs=BEST_BUFS)
    return _NC_CACHE[key]


def kernel(x, cos, sin, Wq, Wk, Wv, Wo):
    x = np.asarray(x, dtype=np.float32)
    cos = np.asarray(cos, dtype=np.float32)
    sin = np.asarray(sin, dtype=np.float32)
    Wq = np.asarray(Wq, dtype=np.float32)
    Wk = np.asarray(Wk, dtype=np.float32)
    Wv = np.asarray(Wv, dtype=np.float32)
    Wo = np.asarray(Wo, dtype=np.float32)
    B, S, HID = x.shape

    in_maps = []
    for i in range(8):
        b, g = i // 4, i % 4
        in_maps.append(make_in_map(
            x[b], cos, sin,
            Wq[:, g * NH * D:(g + 1) * NH * D],
            Wk[:, g * D:(g + 1) * D],
            Wv[:, g * D:(g + 1) * D],
            Wo[g * NH * D:(g + 1) * NH * D, :],
            S, HID))

    nc = _get_nc(S, HID)
    last_err = None
    for _attempt in range(3):
        try:
            res = run_bass_kernel_spmd(nc, in_maps, core_ids=list(range(8)), trace=False)
            break
        except Exception as e:  # flaky NRT_EXEC_UNIT_UNRECOVERABLE seen on first runs
            last_err = e
            import time as _time
            _time.sleep(5.0)
    else:
        raise last_err
    out = np.zeros((B, S, HID), dtype=np.float32)
    for i in range(8):
        b = i // 4
        out[b] += res.results[i]["out"]
    return out


# revision 67
# speedup vs baseline: 1.0351x; 1.0351x over previous
"""Trainium2 Bass kernel for GQA attention (B=2, S=2048, HID=2048, H=16, HKV=4, RoPE, causal).

Sharding: TP=4 over GQA groups (4 Q heads + 1 KV head per core) x DP=2 over batch.
Core i -> (batch = i // 4, group = i % 4). Each core computes a partial output
x @ Wo_shard for its head group; host sums the 4 partials per batch.

v2 pipeline (all matmul operands fp16, fp32 PSUM accumulation):
  - projections per seq chunk c (xt streamed c-major from DRAM), RoPE on DVE,
    PE-transpose q/k -> qT/kT [d, s]; v stays [s, d].
  - attention per 512-query strip, scores computed TRANSPOSED [sk, sq]
    (kT chunk stationary, qT strip moving) so exp (ACT) writes P^T into SBUF
    directly -- no PE P-transposes, no DVE P copies. Causal triangle of the
    diagonal block is zeroed post-exp by a Pool affine_select.
  - softmax row-sums: fp16 DVE accumulation of P^T chunks, then ONE PE
    ones-matmul per (head, strip) that both sums across partitions and
    broadcasts the result to all 128 partitions (HW's gpsimd
    partition_all_reduce measured ~5us/call -- avoid); normalization is
    folded into the PV PSUM->SBUF eviction multiply (DVE).
  - token-level software pipelining: HW measures independent 512-col matmuls
    at ~162ns but PSUM-accumulating chains at ~270ns (RMW bubble), so
    projection matmul pairs are woven between score chunks, and the previous
    strip's O-projection matmuls between PV chunks, hiding both the
    accumulation bubbles and the ACT exp pacing (~1.07us per 512-col chunk).
  - PSUM: qps x2, trps, shared scores/O-proj/rowsum ring x3, pv/kv shared x2
    = 8 banks; fp16 output (host upcasts while summing the 4 TP partials);
    cross-strip pipelining: the next strip's first two score heads are
    emitted during the current strip's tail to prefill the ACT-bound
    final strip.
"""
import sys
sys.path.insert(0, "/opt/trn_rl_repo")
import math
import numpy as np
import concourse.mybir as mybir
import concourse.tile as tile
from concourse import bacc, bass_isa
from concourse.bass_utils import run_bass_kernel_spmd
from concourse.masks import make_identity

F16 = mybir.dt.float16
F32 = mybir.dt.float32
AF = mybir.ActivationFunctionType
ALU = mybir.AluOpType
RED = bass_isa.ReduceOp

NH = 4          # q heads per core
D = 128         # head dim
MASK_VAL = -1e9
EXP_BIAS = -4.0

DEFAULT_BUFS = dict(qps=2, trps=1, scop=3, pv=2,
                    xt=4, q16=2, k16=2, pt=2, racc=2, rsum=2, rcp=2, attnT=2, osb=2)


def build(S=2048, HID=2048, repeat=1, bufs=None, norm=True, out_eng="sp", startup="spread",
          evict="act", racc_pairs=False, unify=False, bias_scalar=False, op_riffle=False,
          pair_prologue=True, out16=True, xstrip=True, spv=False):
    bz = dict(DEFAULT_BUFS)
    if bufs:
        bz.update(bufs)
    SC = S // 128        # seq chunks
    NT = S // 512        # 512-wide query strips
    HC = HID // 128      # hidden (contraction) chunks
    QW = NH * D          # 512: q width per core
    CB = HC * 128        # xt block cols per seq chunk (2048)
    scale = 1.0 / math.sqrt(D)

    nc = bacc.Bacc(None, target_bir_lowering=False, debug=False)
    with tile.TileContext(nc) as tc:
        with tc.tile_pool(name="dram", bufs=1, space="DRAM") as dram:
            # xt c-major: block c holds all HC hid-chunks of seq chunk c
            xt_d = dram.tile([128, SC * CB], F16, kind="ExternalInput", name="xt", uniquify=False)
            wqkv_d = dram.tile([128, HC * (QW + 256)], F16, kind="ExternalInput", name="wqkv", uniquify=False)
            cos_d = dram.tile([128, SC * QW], F16, kind="ExternalInput", name="cos4", uniquify=False)
            sin_d = dram.tile([128, SC * QW], F16, kind="ExternalInput", name="sin4", uniquify=False)
            wo_d = dram.tile([128, NH * HID], F16, kind="ExternalInput", name="wo", uniquify=False)
            out_d = dram.tile([S, HID], F16 if out16 else F32, kind="ExternalOutput", name="out", uniquify=False)

            with tc.tile_pool(name="keep", bufs=1) as keep:
                qT_sb = keep.tile([128, NH * S], F16)   # [d, h*S + s]
                kT_sb = keep.tile([128, S], F16)        # [d, sk]
                v_sb = keep.tile([128, S], F16)         # [sk%128, chunk*128 + d]
                ident = keep.tile([128, 128], F16)
                make_identity(nc, ident[:])
                ones128 = keep.tile([128, 128], F16)
                nc.gpsimd.memset(ones128[:], 1.0)
                ebias = keep.tile([128, 1], F32)
                nc.gpsimd.memset(ebias[:], EXP_BIAS)
                # transposed causal mask for the diagonal 128x128 block:
                # visible (0) iff key_pos (partition) <= query_pos (free)
                cmaskT = keep.tile([128, 128], F32)
                nc.gpsimd.memset(cmaskT[:], 0.0)
                nc.gpsimd.affine_select(
                    out=cmaskT[:], in_=cmaskT[:], compare_op=ALU.is_ge,
                    fill=MASK_VAL, base=0, pattern=[[1, 128]], channel_multiplier=-1,
                )

                from contextlib import ExitStack
                _rep = ExitStack()
                if repeat > 1:
                    _rep.enter_context(tc.For_i(0, repeat, 1))

                with tc.tile_pool(name="pp", bufs=1) as pp, \
                     tc.tile_pool(name="psp", bufs=2, space="PSUM") as psp:
                    # persistent-ish weights (reloaded per repeat iteration)
                    wqkv_sb = pp.tile([128, HC * (QW + 256)], F16)
                    cos_sb = pp.tile([128, SC * QW], F16)
                    sin_sb = pp.tile([128, SC * QW], F16)
                    wo_sb = pp.tile([128, NH * HID], F16)
                    xt_tiles = {}

                    def fetch_x(c, eng_x=None, eng_cs=None):
                        if c >= SC:
                            return
                        xt_tiles[c] = pp.tile([128, CB], F16, tag="xt", bufs=bz["xt"], name=f"xtb{c}")
                        (eng_x or nc.sync).dma_start(out=xt_tiles[c][:], in_=xt_d[:, c * CB:(c + 1) * CB])
                        (eng_cs or nc.sync).dma_start(out=cos_sb[:, c * QW:(c + 1) * QW],
                                                      in_=cos_d[:, c * QW:(c + 1) * QW])
                        (eng_cs or nc.sync).dma_start(out=sin_sb[:, c * QW:(c + 1) * QW],
                                                      in_=sin_d[:, c * QW:(c + 1) * QW])

                    # Startup DMAs spread across idle engines: xt on ACT,
                    # cos/sin on Pool, so nothing queues behind wqkv on SP.
                    for c in range(min(bz["xt"], SC)):
                        if startup == "spread":
                            fetch_x(c, nc.scalar, nc.scalar)
                        else:
                            fetch_x(c)
                    for hh in range(HC):
                        nc.sync.dma_start(out=wqkv_sb[:, hh * (QW + 256):(hh + 1) * (QW + 256)],
                                          in_=wqkv_d[:, hh * (QW + 256):(hh + 1) * (QW + 256)])
                    nc.sync.dma_start(out=wo_sb[:], in_=wo_d[:])

                    def proj_chunk_a(c, state):
                        """First half of projections for seq chunk c."""
                        xt_sb = xt_tiles.pop(c)
                        q_ps = psp.tile([128, QW], F32, tag="qps", bufs=bz["qps"], name=f"qps{c}")
                        kv_ps = psp.tile([128, 256], F32, tag="pv", bufs=bz["pv"], name=f"kvps{c}")
                        state.update(xt_sb=xt_sb, q_ps=q_ps, kv_ps=kv_ps)
                        for hh in range(HC // 2):
                            xk = xt_sb[:, hh * 128:(hh + 1) * 128]
                            nc.tensor.matmul(q_ps[:], xk, wqkv_sb[:, hh * (QW + 256): hh * (QW + 256) + QW],
                                             start=(hh == 0), stop=False)
                            nc.tensor.matmul(kv_ps[:], xk, wqkv_sb[:, hh * (QW + 256) + QW: (hh + 1) * (QW + 256)],
                                             start=(hh == 0), stop=False)

                    def proj_chunk_b(c, state):
                        """Second half of projections for seq chunk c."""
                        xt_sb, q_ps, kv_ps = state["xt_sb"], state["q_ps"], state["kv_ps"]
                        for hh in range(HC // 2, HC):
                            xk = xt_sb[:, hh * 128:(hh + 1) * 128]
                            nc.tensor.matmul(q_ps[:], xk, wqkv_sb[:, hh * (QW + 256): hh * (QW + 256) + QW],
                                             start=False, stop=(hh == HC - 1))
                            nc.tensor.matmul(kv_ps[:], xk, wqkv_sb[:, hh * (QW + 256) + QW: (hh + 1) * (QW + 256)],
                                             start=False, stop=(hh == HC - 1))
                        # prefetch a later chunk into the slot just vacated
                        fetch_x(c + bz["xt"])

                    def proj_chunk(c):
                        st = {}
                        proj_chunk_a(c, st)
                        proj_chunk_b(c, st)
                        proj_rope(c, st)

                    def proj_rope(c, state):
                        """RoPE + transposes for seq chunk c."""
                        q_ps, kv_ps = state["q_ps"], state["kv_ps"]
                        # --- RoPE on q (4 heads batched) ---
                        q4 = q_ps[:].rearrange("p (h d) -> p h d", h=NH)
                        cos4v = cos_sb[:, c * QW:(c + 1) * QW].rearrange("p (h d) -> p h d", h=NH)
                        sin4v = sin_sb[:, c * QW:(c + 1) * QW].rearrange("p (h d) -> p h d", h=NH)
                        rot = pp.tile([128, QW], F32, tag="rot")
                        rot4 = rot[:].rearrange("p (h d) -> p h d", h=NH)
                        nc.vector.tensor_mul(rot4[:, :, 0:64], q4[:, :, 64:128], sin4v[:, :, 0:64])
                        nc.vector.tensor_mul(rot4[:, :, 64:128], q4[:, :, 0:64], sin4v[:, :, 64:128])
                        qc = pp.tile([128, QW], F32, tag="qc")
                        nc.vector.tensor_mul(qc[:], q_ps[:], cos_sb[:, c * QW:(c + 1) * QW])
                        q16 = pp.tile([128, QW], F16, tag="q16", bufs=bz["q16"])
                        nc.vector.tensor_add(q16[:], qc[:], rot[:])
                        # --- RoPE on k (head 0 slices of cos/sin) ---
                        k1 = kv_ps[:, 0:128]
                        cos1 = cos_sb[:, c * QW: c * QW + 128]
                        sin1 = sin_sb[:, c * QW: c * QW + 128]
                        krot = pp.tile([128, 128], F32, tag="krot")
                        nc.vector.tensor_mul(krot[:, 0:64], k1[:, 64:128], sin1[:, 0:64])
                        nc.vector.tensor_mul(krot[:, 64:128], k1[:, 0:64], sin1[:, 64:128])
                        kc_t = pp.tile([128, 128], F32, tag="kc")
                        nc.vector.tensor_mul(kc_t[:], k1, cos1)
                        k16 = pp.tile([128, 128], F16, tag="k16", bufs=bz["k16"])
                        nc.vector.tensor_add(k16[:], kc_t[:], krot[:])
                        # --- v to persistent [s, d] fp16 ---
                        nc.vector.tensor_copy(v_sb[:, c * 128:(c + 1) * 128], kv_ps[:, 128:256])
                        # --- transpose q heads + k into qT/kT ---
                        tr_ps = psp.tile([128, 640], F16, tag="trps", bufs=bz["trps"])
                        for h in range(NH):
                            nc.tensor.transpose(tr_ps[:, h * 128:(h + 1) * 128], q16[:, h * 128:(h + 1) * 128], ident[:])
                        nc.tensor.transpose(tr_ps[:, 512:640], k16[:], ident[:])
                        qT_view = qT_sb[:].rearrange("p (h s) -> p h s", h=NH)[:, :, c * 128:(c + 1) * 128]
                        nc.vector.tensor_copy(qT_view, tr_ps[:, 0:512].rearrange("p (h s) -> p h s", h=NH))
                        nc.vector.tensor_copy(kT_sb[:, c * 128:(c + 1) * 128], tr_ps[:, 512:640])

                    def oproj_group(t, ci, attnT):
                        c = 4 * t + ci
                        osb = pp.tile([128, HID], F16 if out16 else F32, tag="osb", bufs=bz["osb"], name=f"osb{c}")
                        for n in range(HID // 512):
                            op = psp.tile([128, 512], F32, tag="scop", bufs=bz["scop"], name=f"op{c}_{n}")
                            for h in range(NH):
                                nc.tensor.matmul(op[:], attnT[:, h * 512 + ci * 128: h * 512 + (ci + 1) * 128],
                                                 wo_sb[:, h * HID + n * 512: h * HID + (n + 1) * 512],
                                                 start=(h == 0), stop=(h == NH - 1))
                            if evict == "mix":
                                ev = nc.scalar.copy if n % 2 == 0 else nc.vector.tensor_copy
                            else:
                                ev = nc.scalar.copy if evict == "act" else nc.vector.tensor_copy
                            ev(osb[:, n * 512:(n + 1) * 512], op[:])
                        out_e = {"pool": nc.gpsimd, "sp": nc.sync, "act": nc.scalar}[out_eng]
                        out_e.dma_start(out=out_d[c * 128:(c + 1) * 128, :], in_=osb[:])

                    def proj_pair_tokens(c):
                        """Per-hh (q mm, kv mm) pair tokens + final rope token."""
                        st = {}
                        toks = []

                        def pair(hh):
                            if hh == 0:
                                st["xt_sb"] = xt_tiles.pop(c)
                                st["q_ps"] = psp.tile([128, QW], F32, tag="qps", bufs=bz["qps"], name=f"qps{c}")
                                st["kv_ps"] = psp.tile([128, 256], F32, tag="pv", bufs=bz["pv"], name=f"kvps{c}")
                            xk = st["xt_sb"][:, hh * 128:(hh + 1) * 128]
                            nc.tensor.matmul(st["q_ps"][:], xk, wqkv_sb[:, hh * (QW + 256): hh * (QW + 256) + QW],
                                             start=(hh == 0), stop=(hh == HC - 1))
                            nc.tensor.matmul(st["kv_ps"][:], xk, wqkv_sb[:, hh * (QW + 256) + QW: (hh + 1) * (QW + 256)],
                                             start=(hh == 0), stop=(hh == HC - 1))
                            if hh == HC - 1:
                                fetch_x(c + bz["xt"])

                        for hh in range(HC):
                            toks.append(lambda hh=hh: pair(hh))
                        toks.append(lambda: proj_rope(c, st))
                        return toks

                    def oproj_tokens(t, ci, attnT):
                        """Per-(n,h) matmul tokens; evict folded into h==last."""
                        c = 4 * t + ci
                        st = {}
                        toks = []

                        def mmtok(n, h):
                            if n == 0 and h == 0:
                                st["osb"] = pp.tile([128, HID], F16 if out16 else F32, tag="osb", bufs=bz["osb"], name=f"osb{c}")
                            if h == 0:
                                st["op"] = psp.tile([128, 512], F32, tag="scop", bufs=bz["scop"], name=f"op{c}_{n}")
                            nc.tensor.matmul(st["op"][:], attnT[:, h * 512 + ci * 128: h * 512 + (ci + 1) * 128],
                                             wo_sb[:, h * HID + n * 512: h * HID + (n + 1) * 512],
                                             start=(h == 0), stop=(h == NH - 1))
                            if h == NH - 1:
                                if evict == "mix":
                                    ev = nc.scalar.copy if n % 2 == 0 else nc.vector.tensor_copy
                                else:
                                    ev = nc.scalar.copy if evict == "act" else nc.vector.tensor_copy
                                ev(st["osb"][:, n * 512:(n + 1) * 512], st["op"][:])

                        for n in range(HID // 512):
                            for h in range(NH):
                                toks.append(lambda n=n, h=h: mmtok(n, h))

                        def dmatok():
                            out_e = {"pool": nc.gpsimd, "sp": nc.sync, "act": nc.scalar}[out_eng]
                            out_e.dma_start(out=out_d[c * 128:(c + 1) * 128, :], in_=st["osb"][:])
                        toks.append(dmatok)
                        return toks

                    class Feeder:
                        """Drains filler tokens evenly across primary slots."""
                        def __init__(self, toks, slots):
                            self.toks = toks
                            self.per = len(toks) / max(slots, 1)
                            self.acc = 0.0
                            self.i = 0

                        def step(self):
                            self.acc += self.per
                            while self.i < len(self.toks) and self.i < int(self.acc + 1e-9):
                                self.toks[self.i]()
                                self.i += 1

                        def flush(self):
                            while self.i < len(self.toks):
                                self.toks[self.i]()
                                self.i += 1

                    if pair_prologue:
                        # interleave chunk pairs: 4 independent PSUM chains
                        # (q c0, q c1, kv c0, kv c1) hide accumulation bubbles
                        for c0 in (0, 2):
                            la = proj_pair_tokens(c0)
                            lb = proj_pair_tokens(c0 + 1)
                            for x, y in zip(la, lb):
                                x(); y()
                    else:
                        for c in range(4):
                            proj_chunk(c)

                    def make_strip(t, pending_op_toks):
                        nk = 4 * t + 4
                        nxt = [4 * (t + 1) + i for i in range(4)] if t + 1 < NT else []
                        projtoks = [tok for c in nxt for tok in proj_pair_tokens(c)]
                        if not projtoks:
                            # last strip: ACT-paced scores leave PE idle, so
                            # feed O-proj tokens into score slots as well
                            fproj = Feeder(pending_op_toks, NH * nk + NH * (nk + 1))
                            fop = fproj
                            skip_early = True
                        else:
                            fproj = Feeder(projtoks, NH * nk)
                            fop = Feeder(pending_op_toks, NH * (nk + 1))
                            skip_early = False
                        return dict(t=t, nk=nk, fproj=fproj, fop=fop, raccs=[], pts=[],
                                    attnT=None, skip_early=skip_early)

                    def scores_units(Sd, h, early=False):
                        t, nk, fproj = Sd["t"], Sd["nk"], Sd["fproj"]
                        pts, raccs = Sd["pts"], Sd["raccs"]
                        pt_sb = pp.tile([128, nk * 512], F16, tag="pt", bufs=bz["pt"], name=f"pt{t}_{h}")
                        pts.append(pt_sb)
                        racc = pp.tile([128, 512], F16, tag="racc", bufs=bz["racc"], name=f"racc{t}_{h}")
                        raccs.append(racc)
                        units = []

                        def chunk(kc):
                                off = 128 * max(0, kc - 4 * t)
                                w = 512 - off
                                sc = psp.tile([128, 512], F32, tag="scop", bufs=bz["scop"])
                                nc.tensor.matmul(sc[:, off:512], kT_sb[:, kc * 128:(kc + 1) * 128],
                                                 qT_sb[:, h * S + t * 512 + off: h * S + (t + 1) * 512],
                                                 start=True, stop=True)
                                if not early:
                                    fproj.step()
                                nc.scalar.activation(pt_sb[:, kc * 512 + off:(kc + 1) * 512], sc[:, off:512],
                                                     AF.Exp, scale=scale,
                                                     bias=(EXP_BIAS if bias_scalar else ebias[:]))
                                if kc >= 4 * t:
                                    # zero the causal upper triangle of the diagonal
                                    # 128-col block (post-exp) on Pool
                                    nc.gpsimd.affine_select(
                                        out=pt_sb[:, kc * 512 + off: kc * 512 + off + 128],
                                        in_=pt_sb[:, kc * 512 + off: kc * 512 + off + 128],
                                        compare_op=ALU.is_ge, fill=0.0,
                                        base=0, pattern=[[1, 128]], channel_multiplier=-1,
                                    )
                                if off:
                                    nc.gpsimd.memset(pt_sb[:, kc * 512: kc * 512 + off], 0.0)
                                if kc == 0:
                                    nc.vector.tensor_copy(racc[:], pt_sb[:, 0:512])
                                else:
                                    nc.vector.tensor_add(racc[:, off:512], racc[:, off:512],
                                                         pt_sb[:, kc * 512 + off:(kc + 1) * 512])

                        for kc in range(nk):
                            units.append(lambda kc=kc: chunk(kc))
                        return units

                    def scores_head(Sd, h, early=False):
                        for u in scores_units(Sd, h, early):
                            u()
                    def pv_units(Sd, h):
                        t, nk, fop = Sd["t"], Sd["nk"], Sd["fop"]
                        pts, raccs = Sd["pts"], Sd["raccs"]
                        st = {}

                        def chunk(kc):
                            if kc == 0:
                                if Sd["attnT"] is None:
                                    Sd["attnT"] = pp.tile([128, NH * 512], F16, tag="attnT",
                                                          bufs=bz["attnT"], name=f"attnT{t}")
                                if norm:
                                    # ones-matmul: sums racc across partitions AND
                                    # broadcasts to all 128 output partitions
                                    rsum = psp.tile([128, 512], F32, tag="scop", bufs=bz["scop"], name=f"rs{t}_{h}")
                                    nc.tensor.matmul(rsum[:], ones128[:], raccs[h][:], start=True, stop=True)
                                    st["rcp"] = pp.tile([128, 512], F32, tag="rcp", bufs=bz["rcp"], name=f"rcp{t}_{h}")
                                    nc.vector.reciprocal(st["rcp"][:], rsum[:])
                                fop.step()
                                st["pv"] = psp.tile([128, 512], F32, tag="pv", bufs=bz["pv"], name=f"pvt{t}_{h}")
                            nc.tensor.matmul(st["pv"][:], v_sb[:, kc * 128:(kc + 1) * 128],
                                             pts[h][:, kc * 512:(kc + 1) * 512],
                                             start=(kc == 0), stop=(kc == nk - 1))
                            fop.step()
                            if kc == nk - 1:
                                attnT = Sd["attnT"]
                                if norm:
                                    nc.vector.tensor_mul(attnT[:, h * 512:(h + 1) * 512], st["pv"][:], st["rcp"][:])
                                else:
                                    nc.vector.tensor_copy(attnT[:, h * 512:(h + 1) * 512], st["pv"][:])

                        return [lambda kc=kc: chunk(kc) for kc in range(nk)]

                    def pv_head(Sd, h):
                        for u in pv_units(Sd, h):
                            u()

                    def seg(su, pu):
                        i = j = 0
                        while i < len(su) or j < len(pu):
                            if i < len(su):
                                su[i](); i += 1
                            if j < len(pu):
                                pu[j](); j += 1

                    def seg_half(su, pu):
                        a, b = (len(su) + 1) // 2, (len(pu) + 1) // 2
                        for u in su[:a]:
                            u()
                        for u in pu[:b]:
                            u()
                        for u in su[a:]:
                            u()
                        for u in pu[b:]:
                            u()

                    # cross-strip software pipeline: the next strip's first two
                    # score heads (ACT exp work) are emitted during this
                    # strip's tail, where ACT otherwise idles
                    if xstrip and spv == "half":
                        # coarse half-segment interleave: the next score head's
                        # first half lands on ACT before each PV half runs
                        cur = make_strip(0, [])
                        scores_head(cur, 0)
                        scores_head(cur, 1)
                        for t in range(NT):
                            seg_half(scores_units(cur, 2), pv_units(cur, 0))
                            seg_half(scores_units(cur, 3), pv_units(cur, 1))
                            cur["fproj"].flush()
                            cur["fop"].flush()
                            optoks = [tok for ci in range(4)
                                      for tok in oproj_tokens(t, ci, cur["attnT"])]
                            if t + 1 < NT:
                                nxt = make_strip(t + 1, optoks)
                                seg_half(scores_units(nxt, 0, early=nxt["skip_early"]),
                                         pv_units(cur, 2))
                                seg_half(scores_units(nxt, 1, early=nxt["skip_early"]),
                                         pv_units(cur, 3))
                                cur = nxt
                            else:
                                pv_head(cur, 2)
                                pv_head(cur, 3)
                                for tok in optoks:
                                    tok()
                    elif xstrip and spv:
                        # head h+1 score chunks interleave with head h PV
                        # chunks so ACT's exp stream never starves between
                        # heads (and PV accumulation bubbles hide behind
                        # independent score matmuls)
                        cur = make_strip(0, [])
                        scores_head(cur, 0)
                        scores_head(cur, 1)
                        for t in range(NT):
                            seg(scores_units(cur, 2), pv_units(cur, 0))
                            seg(scores_units(cur, 3), pv_units(cur, 1))
                            cur["fproj"].flush()
                            cur["fop"].flush()
                            optoks = [tok for ci in range(4)
                                      for tok in oproj_tokens(t, ci, cur["attnT"])]
                            if t + 1 < NT:
                                nxt = make_strip(t + 1, optoks)
                                seg(scores_units(nxt, 0, early=nxt["skip_early"]),
                                    pv_units(cur, 2))
                                seg(scores_units(nxt, 1, early=nxt["skip_early"]),
                                    pv_units(cur, 3))
                                cur = nxt
                            else:
                                pv_head(cur, 2)
                                pv_head(cur, 3)
                                for tok in optoks:
                                    tok()
                    elif xstrip:
                        cur = make_strip(0, [])
                        scores_head(cur, 0)
                        scores_head(cur, 1)
                        for t in range(NT):
                            pv_head(cur, 0)
                            scores_head(cur, 2)
                            pv_head(cur, 1)
                            scores_head(cur, 3)
                            pv_head(cur, 2)
                            cur["fproj"].flush()
                            cur["fop"].flush()
                            optoks = [tok for ci in range(4)
                                      for tok in oproj_tokens(t, ci, cur["attnT"])]
                            if t + 1 < NT:
                                nxt = make_strip(t + 1, optoks)
                                scores_head(nxt, 0, early=nxt["skip_early"])
                                pv_head(cur, 3)
                                scores_head(nxt, 1, early=nxt["skip_early"])
                                cur = nxt
                            else:
                                pv_head(cur, 3)
                                if op_riffle:
                                    # tail drain: riffle ci pairs so adjacent
                                    # O-proj matmuls alternate PSUM banks (the
                                    # scop ring is free of scores here)
                                    n4 = len(optoks) // 4
                                    for b0 in range(0, 4, 2):
                                        la = optoks[b0 * n4:(b0 + 1) * n4]
                                        lb = optoks[(b0 + 1) * n4:(b0 + 2) * n4]
                                        for x, y in zip(la, lb):
                                            x(); y()
                                else:
                                    for tok in optoks:
                                        tok()
                    else:
                        pending = []
                        for t in range(NT):
                            Sd = make_strip(t, pending)
                            scores_head(Sd, 0)
                            scores_head(Sd, 1)
                            pv_head(Sd, 0)
                            scores_head(Sd, 2)
                            pv_head(Sd, 1)
                            scores_head(Sd, 3)
                            pv_head(Sd, 2)
                            pv_head(Sd, 3)
                            Sd["fproj"].flush()
                            Sd["fop"].flush()
                            pending = [tok for ci in range(4)
                                       for tok in oproj_tokens(t, ci, Sd["attnT"])]
                        for tok in pending:
                            tok()
                _rep.close()
    nc.compile()
    return nc


def _chunk_major(a, rows=128):
    """[R, C] -> [128, (R//128)*C] with row-chunk-major free layout."""
    r, c = a.shape
    return np.ascontiguousarray(a.reshape(r // rows, rows, c).transpose(1, 0, 2).reshape(rows, (r // rows) * c))


def make_in_map(x_b, cos, sin, wq_g, wk_g, wv_g, wo_g, S, HID):
    SC = S // 128
    HC = HID // 128
    # xt c-major: xt[p, c*HC*128 + hh*128 + f] = x_b.T[hh*128+p, c*128+f]
    xT = np.ascontiguousarray(x_b.T).astype(np.float16)  # [HID, S]
    xt = xT.reshape(HC, 128, SC, 128).transpose(1, 2, 0, 3).reshape(128, SC * HC * 128)
    wqkv = _chunk_major(np.concatenate([wq_g, wk_g, wv_g], axis=1)).astype(np.float16)
    cosr = cos[:S].reshape(SC, 128, D)
    cos4 = np.repeat(cosr[:, :, None, :], NH, axis=2).transpose(1, 0, 2, 3).reshape(128, SC * NH * D)
    sing = np.concatenate([-sin[:S, :64], sin[:S, 64:]], axis=1).reshape(SC, 128, D)
    sin4 = np.repeat(sing[:, :, None, :], NH, axis=2).transpose(1, 0, 2, 3).reshape(128, SC * NH * D)
    wo = _chunk_major(wo_g).astype(np.float16)
    return {
        "xt": np.ascontiguousarray(xt),
        "wqkv": wqkv,
        "cos4": np.ascontiguousarray(cos4).astype(np.float16),
        "sin4": np.ascontiguousarray(sin4).astype(np.float16),
        "wo": wo,
    }


_NC_CACHE = {}

# best-measured configuration (updated as experiments conclude)
BEST_BUFS = dict(DEFAULT_BUFS)


def _get_nc(S, HID):
    key = (S, HID)
    if key not in _NC_CACHE:
        _NC_CACHE[key] = build(S, HID, bufs=BEST_BUFS)
    return _NC_CACHE[key]


def kernel(x, cos, sin, Wq, Wk, Wv, Wo):
    x = np.asarray(x, dtype=np.float32)
    cos = np.asarray(cos, dtype=np.float32)
    sin = np.asarray(sin, dtype=np.float32)
    Wq = np.asarray(Wq, dtype=np.float32)
    Wk = np.asarray(Wk, dtype=np.float32)
    Wv = np.asarray(Wv, dtype=np.float32)
    Wo = np.asarray(Wo, dtype=np.float32)
    B, S, HID = x.shape

    in_maps = []
    for i in range(8):
        b, g = i // 4, i % 4
        in_maps.append(make_in_map(
            x[b], cos, sin,
            Wq[:, g * NH * D:(g + 1) * NH * D],
            Wk[:, g * D:(g + 1) * D],
            Wv[:, g * D:(g + 1) * D],
            Wo[g * NH * D:(g + 1) * NH * D, :],
            S, HID))

    nc = _get_nc(S, HID)
    last_err = None
    for _attempt in range(3):
        try:
            res = run_bass_kernel_spmd(nc, in_maps, core_ids=list(range(8)), trace=False)
            break
        except Exception as e:  # flaky NRT_EXEC_UNIT_UNRECOVERABLE seen on first runs
            last_err = e
            import time as _time
            _time.sleep(5.0)
    else:
        raise last_err
    out = np.zeros((B, S, HID), dtype=np.float32)
    for i in range(8):
        b = i // 4
        out[b] += res.results[i]["out"]
    return out


# revision 68
# speedup vs baseline: 1.0379x; 1.0027x over previous
"""Trainium2 Bass kernel for GQA attention (B=2, S=2048, HID=2048, H=16, HKV=4, RoPE, causal).

Sharding: TP=4 over GQA groups (4 Q heads + 1 KV head per core) x DP=2 over batch.
Core i -> (batch = i // 4, group = i % 4). Each core computes a partial output
x @ Wo_shard for its head group; host sums the 4 partials per batch.

v2 pipeline (all matmul operands fp16, fp32 PSUM accumulation):
  - projections per seq chunk c (xt streamed c-major from DRAM), RoPE on DVE,
    PE-transpose q/k -> qT/kT [d, s]; v stays [s, d].
  - attention per 512-query strip, scores computed TRANSPOSED [sk, sq]
    (kT chunk stationary, qT strip moving) so exp (ACT) writes P^T into SBUF
    directly -- no PE P-transposes, no DVE P copies. Causal triangle of the
    diagonal block is zeroed post-exp by a Pool affine_select.
  - softmax row-sums: fp16 DVE accumulation of P^T chunks, then ONE PE
    ones-matmul per (head, strip) that both sums across partitions and
    broadcasts the result to all 128 partitions (HW's gpsimd
    partition_all_reduce measured ~5us/call -- avoid); normalization is
    folded into the PV PSUM->SBUF eviction multiply (DVE).
  - token-level software pipelining: HW measures independent 512-col matmuls
    at ~162ns but PSUM-accumulating chains at ~270ns (RMW bubble), so
    projection matmul pairs are woven between score chunks, and the previous
    strip's O-projection matmuls between PV chunks, hiding both the
    accumulation bubbles and the ACT exp pacing (~1.07us per 512-col chunk).
  - PSUM: qps x2, trps, shared scores/O-proj/rowsum ring x3, pv/kv shared x2
    = 8 banks; fp16 output (host upcasts while summing the 4 TP partials);
    cross-strip pipelining: the next strip's first two score heads are
    emitted during the current strip's tail to prefill the ACT-bound
    final strip.
"""
import sys
sys.path.insert(0, "/opt/trn_rl_repo")
import math
import numpy as np
import concourse.mybir as mybir
import concourse.tile as tile
from concourse import bacc, bass_isa
from concourse.bass_utils import run_bass_kernel_spmd
from concourse.masks import make_identity

F16 = mybir.dt.float16
F32 = mybir.dt.float32
AF = mybir.ActivationFunctionType
ALU = mybir.AluOpType
RED = bass_isa.ReduceOp

NH = 4          # q heads per core
D = 128         # head dim
MASK_VAL = -1e9
EXP_BIAS = -4.0

DEFAULT_BUFS = dict(qps=2, trps=1, scop=3, pv=2,
                    xt=4, q16=3, k16=3, pt=2, racc=2, rsum=2, rcp=3, attnT=3, osb=3)


def build(S=2048, HID=2048, repeat=1, bufs=None, norm=True, out_eng="sp", startup="spread",
          evict="act", racc_pairs=False, unify=False, bias_scalar=False, op_riffle=False,
          pair_prologue=True, out16=True, xstrip=True, spv=False):
    bz = dict(DEFAULT_BUFS)
    if bufs:
        bz.update(bufs)
    SC = S // 128        # seq chunks
    NT = S // 512        # 512-wide query strips
    HC = HID // 128      # hidden (contraction) chunks
    QW = NH * D          # 512: q width per core
    CB = HC * 128        # xt block cols per seq chunk (2048)
    scale = 1.0 / math.sqrt(D)

    nc = bacc.Bacc(None, target_bir_lowering=False, debug=False)
    with tile.TileContext(nc) as tc:
        with tc.tile_pool(name="dram", bufs=1, space="DRAM") as dram:
            # xt c-major: block c holds all HC hid-chunks of seq chunk c
            xt_d = dram.tile([128, SC * CB], F16, kind="ExternalInput", name="xt", uniquify=False)
            wqkv_d = dram.tile([128, HC * (QW + 256)], F16, kind="ExternalInput", name="wqkv", uniquify=False)
            cos_d = dram.tile([128, SC * QW], F16, kind="ExternalInput", name="cos4", uniquify=False)
            sin_d = dram.tile([128, SC * QW], F16, kind="ExternalInput", name="sin4", uniquify=False)
            wo_d = dram.tile([128, NH * HID], F16, kind="ExternalInput", name="wo", uniquify=False)
            out_d = dram.tile([S, HID], F16 if out16 else F32, kind="ExternalOutput", name="out", uniquify=False)

            with tc.tile_pool(name="keep", bufs=1) as keep:
                qT_sb = keep.tile([128, NH * S], F16)   # [d, h*S + s]
                kT_sb = keep.tile([128, S], F16)        # [d, sk]
                v_sb = keep.tile([128, S], F16)         # [sk%128, chunk*128 + d]
                ident = keep.tile([128, 128], F16)
                make_identity(nc, ident[:])
                ones128 = keep.tile([128, 128], F16)
                nc.gpsimd.memset(ones128[:], 1.0)
                ebias = keep.tile([128, 1], F32)
                nc.gpsimd.memset(ebias[:], EXP_BIAS)
                # transposed causal mask for the diagonal 128x128 block:
                # visible (0) iff key_pos (partition) <= query_pos (free)
                cmaskT = keep.tile([128, 128], F32)
                nc.gpsimd.memset(cmaskT[:], 0.0)
                nc.gpsimd.affine_select(
                    out=cmaskT[:], in_=cmaskT[:], compare_op=ALU.is_ge,
                    fill=MASK_VAL, base=0, pattern=[[1, 128]], channel_multiplier=-1,
                )

                from contextlib import ExitStack
                _rep = ExitStack()
                if repeat > 1:
                    _rep.enter_context(tc.For_i(0, repeat, 1))

                with tc.tile_pool(name="pp", bufs=1) as pp, \
                     tc.tile_pool(name="psp", bufs=2, space="PSUM") as psp:
                    # persistent-ish weights (reloaded per repeat iteration)
                    wqkv_sb = pp.tile([128, HC * (QW + 256)], F16)
                    cos_sb = pp.tile([128, SC * QW], F16)
                    sin_sb = pp.tile([128, SC * QW], F16)
                    wo_sb = pp.tile([128, NH * HID], F16)
                    xt_tiles = {}

                    def fetch_x(c, eng_x=None, eng_cs=None):
                        if c >= SC:
                            return
                        xt_tiles[c] = pp.tile([128, CB], F16, tag="xt", bufs=bz["xt"], name=f"xtb{c}")
                        (eng_x or nc.sync).dma_start(out=xt_tiles[c][:], in_=xt_d[:, c * CB:(c + 1) * CB])
                        (eng_cs or nc.sync).dma_start(out=cos_sb[:, c * QW:(c + 1) * QW],
                                                      in_=cos_d[:, c * QW:(c + 1) * QW])
                        (eng_cs or nc.sync).dma_start(out=sin_sb[:, c * QW:(c + 1) * QW],
                                                      in_=sin_d[:, c * QW:(c + 1) * QW])

                    # Startup DMAs spread across idle engines: xt on ACT,
                    # cos/sin on Pool, so nothing queues behind wqkv on SP.
                    for c in range(min(bz["xt"], SC)):
                        if startup == "spread":
                            fetch_x(c, nc.scalar, nc.scalar)
                        else:
                            fetch_x(c)
                    for hh in range(HC):
                        nc.sync.dma_start(out=wqkv_sb[:, hh * (QW + 256):(hh + 1) * (QW + 256)],
                                          in_=wqkv_d[:, hh * (QW + 256):(hh + 1) * (QW + 256)])
                    nc.sync.dma_start(out=wo_sb[:], in_=wo_d[:])

                    def proj_chunk_a(c, state):
                        """First half of projections for seq chunk c."""
                        xt_sb = xt_tiles.pop(c)
                        q_ps = psp.tile([128, QW], F32, tag="qps", bufs=bz["qps"], name=f"qps{c}")
                        kv_ps = psp.tile([128, 256], F32, tag="pv", bufs=bz["pv"], name=f"kvps{c}")
                        state.update(xt_sb=xt_sb, q_ps=q_ps, kv_ps=kv_ps)
                        for hh in range(HC // 2):
                            xk = xt_sb[:, hh * 128:(hh + 1) * 128]
                            nc.tensor.matmul(q_ps[:], xk, wqkv_sb[:, hh * (QW + 256): hh * (QW + 256) + QW],
                                             start=(hh == 0), stop=False)
                            nc.tensor.matmul(kv_ps[:], xk, wqkv_sb[:, hh * (QW + 256) + QW: (hh + 1) * (QW + 256)],
                                             start=(hh == 0), stop=False)

                    def proj_chunk_b(c, state):
                        """Second half of projections for seq chunk c."""
                        xt_sb, q_ps, kv_ps = state["xt_sb"], state["q_ps"], state["kv_ps"]
                        for hh in range(HC // 2, HC):
                            xk = xt_sb[:, hh * 128:(hh + 1) * 128]
                            nc.tensor.matmul(q_ps[:], xk, wqkv_sb[:, hh * (QW + 256): hh * (QW + 256) + QW],
                                             start=False, stop=(hh == HC - 1))
                            nc.tensor.matmul(kv_ps[:], xk, wqkv_sb[:, hh * (QW + 256) + QW: (hh + 1) * (QW + 256)],
                                             start=False, stop=(hh == HC - 1))
                        # prefetch a later chunk into the slot just vacated
                        fetch_x(c + bz["xt"])

                    def proj_chunk(c):
                        st = {}
                        proj_chunk_a(c, st)
                        proj_chunk_b(c, st)
                        proj_rope(c, st)

                    def proj_rope(c, state):
                        """RoPE + transposes for seq chunk c."""
                        q_ps, kv_ps = state["q_ps"], state["kv_ps"]
                        # --- RoPE on q (4 heads batched) ---
                        q4 = q_ps[:].rearrange("p (h d) -> p h d", h=NH)
                        cos4v = cos_sb[:, c * QW:(c + 1) * QW].rearrange("p (h d) -> p h d", h=NH)
                        sin4v = sin_sb[:, c * QW:(c + 1) * QW].rearrange("p (h d) -> p h d", h=NH)
                        rot = pp.tile([128, QW], F32, tag="rot")
                        rot4 = rot[:].rearrange("p (h d) -> p h d", h=NH)
                        nc.vector.tensor_mul(rot4[:, :, 0:64], q4[:, :, 64:128], sin4v[:, :, 0:64])
                        nc.vector.tensor_mul(rot4[:, :, 64:128], q4[:, :, 0:64], sin4v[:, :, 64:128])
                        qc = pp.tile([128, QW], F32, tag="qc")
                        nc.vector.tensor_mul(qc[:], q_ps[:], cos_sb[:, c * QW:(c + 1) * QW])
                        q16 = pp.tile([128, QW], F16, tag="q16", bufs=bz["q16"])
                        nc.vector.tensor_add(q16[:], qc[:], rot[:])
                        # --- RoPE on k (head 0 slices of cos/sin) ---
                        k1 = kv_ps[:, 0:128]
                        cos1 = cos_sb[:, c * QW: c * QW + 128]
                        sin1 = sin_sb[:, c * QW: c * QW + 128]
                        krot = pp.tile([128, 128], F32, tag="krot")
                        nc.vector.tensor_mul(krot[:, 0:64], k1[:, 64:128], sin1[:, 0:64])
                        nc.vector.tensor_mul(krot[:, 64:128], k1[:, 0:64], sin1[:, 64:128])
                        kc_t = pp.tile([128, 128], F32, tag="kc")
                        nc.vector.tensor_mul(kc_t[:], k1, cos1)
                        k16 = pp.tile([128, 128], F16, tag="k16", bufs=bz["k16"])
                        nc.vector.tensor_add(k16[:], kc_t[:], krot[:])
                        # --- v to persistent [s, d] fp16 ---
                        nc.vector.tensor_copy(v_sb[:, c * 128:(c + 1) * 128], kv_ps[:, 128:256])
                        # --- transpose q heads + k into qT/kT ---
                        tr_ps = psp.tile([128, 640], F16, tag="trps", bufs=bz["trps"])
                        for h in range(NH):
                            nc.tensor.transpose(tr_ps[:, h * 128:(h + 1) * 128], q16[:, h * 128:(h + 1) * 128], ident[:])
                        nc.tensor.transpose(tr_ps[:, 512:640], k16[:], ident[:])
                        qT_view = qT_sb[:].rearrange("p (h s) -> p h s", h=NH)[:, :, c * 128:(c + 1) * 128]
                        nc.vector.tensor_copy(qT_view, tr_ps[:, 0:512].rearrange("p (h s) -> p h s", h=NH))
                        nc.vector.tensor_copy(kT_sb[:, c * 128:(c + 1) * 128], tr_ps[:, 512:640])

                    def oproj_group(t, ci, attnT):
                        c = 4 * t + ci
                        osb = pp.tile([128, HID], F16 if out16 else F32, tag="osb", bufs=bz["osb"], name=f"osb{c}")
                        for n in range(HID // 512):
                            op = psp.tile([128, 512], F32, tag="scop", bufs=bz["scop"], name=f"op{c}_{n}")
                            for h in range(NH):
                                nc.tensor.matmul(op[:], attnT[:, h * 512 + ci * 128: h * 512 + (ci + 1) * 128],
                                                 wo_sb[:, h * HID + n * 512: h * HID + (n + 1) * 512],
                                                 start=(h == 0), stop=(h == NH - 1))
                            if evict == "mix":
                                ev = nc.scalar.copy if n % 2 == 0 else nc.vector.tensor_copy
                            else:
                                ev = nc.scalar.copy if evict == "act" else nc.vector.tensor_copy
                            ev(osb[:, n * 512:(n + 1) * 512], op[:])
                        out_e = {"pool": nc.gpsimd, "sp": nc.sync, "act": nc.scalar}[out_eng]
                        out_e.dma_start(out=out_d[c * 128:(c + 1) * 128, :], in_=osb[:])

                    def proj_pair_tokens(c):
                        """Per-hh (q mm, kv mm) pair tokens + final rope token."""
                        st = {}
                        toks = []

                        def pair(hh):
                            if hh == 0:
                                st["xt_sb"] = xt_tiles.pop(c)
                                st["q_ps"] = psp.tile([128, QW], F32, tag="qps", bufs=bz["qps"], name=f"qps{c}")
                                st["kv_ps"] = psp.tile([128, 256], F32, tag="pv", bufs=bz["pv"], name=f"kvps{c}")
                            xk = st["xt_sb"][:, hh * 128:(hh + 1) * 128]
                            nc.tensor.matmul(st["q_ps"][:], xk, wqkv_sb[:, hh * (QW + 256): hh * (QW + 256) + QW],
                                             start=(hh == 0), stop=(hh == HC - 1))
                            nc.tensor.matmul(st["kv_ps"][:], xk, wqkv_sb[:, hh * (QW + 256) + QW: (hh + 1) * (QW + 256)],
                                             start=(hh == 0), stop=(hh == HC - 1))
                            if hh == HC - 1:
                                fetch_x(c + bz["xt"])

                        for hh in range(HC):
                            toks.append(lambda hh=hh: pair(hh))
                        toks.append(lambda: proj_rope(c, st))
                        return toks

                    def oproj_tokens(t, ci, attnT):
                        """Per-(n,h) matmul tokens; evict folded into h==last."""
                        c = 4 * t + ci
                        st = {}
                        toks = []

                        def mmtok(n, h):
                            if n == 0 and h == 0:
                                st["osb"] = pp.tile([128, HID], F16 if out16 else F32, tag="osb", bufs=bz["osb"], name=f"osb{c}")
                            if h == 0:
                                st["op"] = psp.tile([128, 512], F32, tag="scop", bufs=bz["scop"], name=f"op{c}_{n}")
                            nc.tensor.matmul(st["op"][:], attnT[:, h * 512 + ci * 128: h * 512 + (ci + 1) * 128],
                                             wo_sb[:, h * HID + n * 512: h * HID + (n + 1) * 512],
                                             start=(h == 0), stop=(h == NH - 1))
                            if h == NH - 1:
                                if evict == "mix":
                                    ev = nc.scalar.copy if n % 2 == 0 else nc.vector.tensor_copy
                                else:
                                    ev = nc.scalar.copy if evict == "act" else nc.vector.tensor_copy
                                ev(st["osb"][:, n * 512:(n + 1) * 512], st["op"][:])

                        for n in range(HID // 512):
                            for h in range(NH):
                                toks.append(lambda n=n, h=h: mmtok(n, h))

                        def dmatok():
                            out_e = {"pool": nc.gpsimd, "sp": nc.sync, "act": nc.scalar}[out_eng]
                            out_e.dma_start(out=out_d[c * 128:(c + 1) * 128, :], in_=st["osb"][:])
                        toks.append(dmatok)
                        return toks

                    class Feeder:
                        """Drains filler tokens evenly across primary slots."""
                        def __init__(self, toks, slots):
                            self.toks = toks
                            self.per = len(toks) / max(slots, 1)
                            self.acc = 0.0
                            self.i = 0

                        def step(self):
                            self.acc += self.per
                            while self.i < len(self.toks) and self.i < int(self.acc + 1e-9):
                                self.toks[self.i]()
                                self.i += 1

                        def flush(self):
                            while self.i < len(self.toks):
                                self.toks[self.i]()
                                self.i += 1

                    if pair_prologue:
                        # interleave chunk pairs: 4 independent PSUM chains
                        # (q c0, q c1, kv c0, kv c1) hide accumulation bubbles
                        for c0 in (0, 2):
                            la = proj_pair_tokens(c0)
                            lb = proj_pair_tokens(c0 + 1)
                            for x, y in zip(la, lb):
                                x(); y()
                    else:
                        for c in range(4):
                            proj_chunk(c)

                    def make_strip(t, pending_op_toks):
                        nk = 4 * t + 4
                        nxt = [4 * (t + 1) + i for i in range(4)] if t + 1 < NT else []
                        projtoks = [tok for c in nxt for tok in proj_pair_tokens(c)]
                        if not projtoks:
                            # last strip: ACT-paced scores leave PE idle, so
                            # feed O-proj tokens into score slots as well
                            fproj = Feeder(pending_op_toks, NH * nk + NH * (nk + 1))
                            fop = fproj
                            skip_early = True
                        else:
                            fproj = Feeder(projtoks, NH * nk)
                            fop = Feeder(pending_op_toks, NH * (nk + 1))
                            skip_early = False
                        return dict(t=t, nk=nk, fproj=fproj, fop=fop, raccs=[], pts=[],
                                    attnT=None, skip_early=skip_early)

                    def scores_units(Sd, h, early=False):
                        t, nk, fproj = Sd["t"], Sd["nk"], Sd["fproj"]
                        pts, raccs = Sd["pts"], Sd["raccs"]
                        pt_sb = pp.tile([128, nk * 512], F16, tag="pt", bufs=bz["pt"], name=f"pt{t}_{h}")
                        pts.append(pt_sb)
                        racc = pp.tile([128, 512], F16, tag="racc", bufs=bz["racc"], name=f"racc{t}_{h}")
                        raccs.append(racc)
                        units = []

                        def chunk(kc):
                                off = 128 * max(0, kc - 4 * t)
                                w = 512 - off
                                sc = psp.tile([128, 512], F32, tag="scop", bufs=bz["scop"])
                                nc.tensor.matmul(sc[:, off:512], kT_sb[:, kc * 128:(kc + 1) * 128],
                                                 qT_sb[:, h * S + t * 512 + off: h * S + (t + 1) * 512],
                                                 start=True, stop=True)
                                if not early:
                                    fproj.step()
                                nc.scalar.activation(pt_sb[:, kc * 512 + off:(kc + 1) * 512], sc[:, off:512],
                                                     AF.Exp, scale=scale,
                                                     bias=(EXP_BIAS if bias_scalar else ebias[:]))
                                if kc >= 4 * t:
                                    # zero the causal upper triangle of the diagonal
                                    # 128-col block (post-exp) on Pool
                                    nc.gpsimd.affine_select(
                                        out=pt_sb[:, kc * 512 + off: kc * 512 + off + 128],
                                        in_=pt_sb[:, kc * 512 + off: kc * 512 + off + 128],
                                        compare_op=ALU.is_ge, fill=0.0,
                                        base=0, pattern=[[1, 128]], channel_multiplier=-1,
                                    )
                                if off:
                                    nc.gpsimd.memset(pt_sb[:, kc * 512: kc * 512 + off], 0.0)
                                if kc == 0:
                                    nc.vector.tensor_copy(racc[:], pt_sb[:, 0:512])
                                else:
                                    nc.vector.tensor_add(racc[:, off:512], racc[:, off:512],
                                                         pt_sb[:, kc * 512 + off:(kc + 1) * 512])

                        for kc in range(nk):
                            units.append(lambda kc=kc: chunk(kc))
                        return units

                    def scores_head(Sd, h, early=False):
                        for u in scores_units(Sd, h, early):
                            u()
                    def pv_units(Sd, h):
                        t, nk, fop = Sd["t"], Sd["nk"], Sd["fop"]
                        pts, raccs = Sd["pts"], Sd["raccs"]
                        st = {}

                        def chunk(kc):
                            if kc == 0:
                                if Sd["attnT"] is None:
                                    Sd["attnT"] = pp.tile([128, NH * 512], F16, tag="attnT",
                                                          bufs=bz["attnT"], name=f"attnT{t}")
                                if norm:
                                    # ones-matmul: sums racc across partitions AND
                                    # broadcasts to all 128 output partitions
                                    rsum = psp.tile([128, 512], F32, tag="scop", bufs=bz["scop"], name=f"rs{t}_{h}")
                                    nc.tensor.matmul(rsum[:], ones128[:], raccs[h][:], start=True, stop=True)
                                    st["rcp"] = pp.tile([128, 512], F32, tag="rcp", bufs=bz["rcp"], name=f"rcp{t}_{h}")
                                    nc.vector.reciprocal(st["rcp"][:], rsum[:])
                                fop.step()
                                st["pv"] = psp.tile([128, 512], F32, tag="pv", bufs=bz["pv"], name=f"pvt{t}_{h}")
                            nc.tensor.matmul(st["pv"][:], v_sb[:, kc * 128:(kc + 1) * 128],
                                             pts[h][:, kc * 512:(kc + 1) * 512],
                                             start=(kc == 0), stop=(kc == nk - 1))
                            fop.step()
                            if kc == nk - 1:
                                attnT = Sd["attnT"]
                                if norm:
                                    nc.vector.tensor_mul(attnT[:, h * 512:(h + 1) * 512], st["pv"][:], st["rcp"][:])
                                else:
                                    nc.vector.tensor_copy(attnT[:, h * 512:(h + 1) * 512], st["pv"][:])

                        return [lambda kc=kc: chunk(kc) for kc in range(nk)]

                    def pv_head(Sd, h):
                        for u in pv_units(Sd, h):
                            u()

                    def seg(su, pu):
                        i = j = 0
                        while i < len(su) or j < len(pu):
                            if i < len(su):
                                su[i](); i += 1
                            if j < len(pu):
                                pu[j](); j += 1

                    def seg_half(su, pu):
                        a, b = (len(su) + 1) // 2, (len(pu) + 1) // 2
                        for u in su[:a]:
                            u()
                        for u in pu[:b]:
                            u()
                        for u in su[a:]:
                            u()
                        for u in pu[b:]:
                            u()

                    # cross-strip software pipeline: the next strip's first two
                    # score heads (ACT exp work) are emitted during this
                    # strip's tail, where ACT otherwise idles
                    if xstrip and spv == "half":
                        # coarse half-segment interleave: the next score head's
                        # first half lands on ACT before each PV half runs
                        cur = make_strip(0, [])
                        scores_head(cur, 0)
                        scores_head(cur, 1)
                        for t in range(NT):
                            seg_half(scores_units(cur, 2), pv_units(cur, 0))
                            seg_half(scores_units(cur, 3), pv_units(cur, 1))
                            cur["fproj"].flush()
                            cur["fop"].flush()
                            optoks = [tok for ci in range(4)
                                      for tok in oproj_tokens(t, ci, cur["attnT"])]
                            if t + 1 < NT:
                                nxt = make_strip(t + 1, optoks)
                                seg_half(scores_units(nxt, 0, early=nxt["skip_early"]),
                                         pv_units(cur, 2))
                                seg_half(scores_units(nxt, 1, early=nxt["skip_early"]),
                                         pv_units(cur, 3))
                                cur = nxt
                            else:
                                pv_head(cur, 2)
                                pv_head(cur, 3)
                                for tok in optoks:
                                    tok()
                    elif xstrip and spv:
                        # head h+1 score chunks interleave with head h PV
                        # chunks so ACT's exp stream never starves between
                        # heads (and PV accumulation bubbles hide behind
                        # independent score matmuls)
                        cur = make_strip(0, [])
                        scores_head(cur, 0)
                        scores_head(cur, 1)
                        for t in range(NT):
                            seg(scores_units(cur, 2), pv_units(cur, 0))
                            seg(scores_units(cur, 3), pv_units(cur, 1))
                            cur["fproj"].flush()
                            cur["fop"].flush()
                            optoks = [tok for ci in range(4)
                                      for tok in oproj_tokens(t, ci, cur["attnT"])]
                            if t + 1 < NT:
                                nxt = make_strip(t + 1, optoks)
                                seg(scores_units(nxt, 0, early=nxt["skip_early"]),
                                    pv_units(cur, 2))
                                seg(scores_units(nxt, 1, early=nxt["skip_early"]),
                                    pv_units(cur, 3))
                                cur = nxt
                            else:
                                pv_head(cur, 2)
                                pv_head(cur, 3)
                                for tok in optoks:
                                    tok()
                    elif xstrip:
                        cur = make_strip(0, [])
                        scores_head(cur, 0)
                        scores_head(cur, 1)
                        for t in range(NT):
                            pv_head(cur, 0)
                            scores_head(cur, 2)
                            pv_head(cur, 1)
                            scores_head(cur, 3)
                            pv_head(cur, 2)
                            cur["fproj"].flush()
                            cur["fop"].flush()
                            optoks = [tok for ci in range(4)
                                      for tok in oproj_tokens(t, ci, cur["attnT"])]
                            if t + 1 < NT:
                                nxt = make_strip(t + 1, optoks)
                                scores_head(nxt, 0, early=nxt["skip_early"])
                                pv_head(cur, 3)
                                scores_head(nxt, 1, early=nxt["skip_early"])
                                cur = nxt
                            else:
                                pv_head(cur, 3)
                                if op_riffle:
                                    # tail drain: riffle ci pairs so adjacent
                                    # O-proj matmuls alternate PSUM banks (the
                                    # scop ring is free of scores here)
                                    n4 = len(optoks) // 4
                                    for b0 in range(0, 4, 2):
                                        la = optoks[b0 * n4:(b0 + 1) * n4]
                                        lb = optoks[(b0 + 1) * n4:(b0 + 2) * n4]
                                        for x, y in zip(la, lb):
                                            x(); y()
                                else:
                                    for tok in optoks:
                                        tok()
                    else:
                        pending = []
                        for t in range(NT):
                            Sd = make_strip(t, pending)
                            scores_head(Sd, 0)
                            scores_head(Sd, 1)
                            pv_head(Sd, 0)
                            scores_head(Sd, 2)
                            pv_head(Sd, 1)
                            scores_head(Sd, 3)
                            pv_head(Sd, 2)
                            pv_head(Sd, 3)
                            Sd["fproj"].flush()
                            Sd["fop"].flush()
                            pending = [tok for ci in range(4)
                                       for tok in oproj_tokens(t, ci, Sd["attnT"])]
                        for tok in pending:
                            tok()
                _rep.close()
    nc.compile()
    return nc


def _chunk_major(a, rows=128):
    """[R, C] -> [128, (R//128)*C] with row-chunk-major free layout."""
    r, c = a.shape
    return np.ascontiguousarray(a.reshape(r // rows, rows, c).transpose(1, 0, 2).reshape(rows, (r // rows) * c))


def make_in_map(x_b, cos, sin, wq_g, wk_g, wv_g, wo_g, S, HID):
    SC = S // 128
    HC = HID // 128
    # xt c-major: xt[p, c*HC*128 + hh*128 + f] = x_b.T[hh*128+p, c*128+f]
    xT = np.ascontiguousarray(x_b.T).astype(np.float16)  # [HID, S]
    xt = xT.reshape(HC, 128, SC, 128).transpose(1, 2, 0, 3).reshape(128, SC * HC * 128)
    wqkv = _chunk_major(np.concatenate([wq_g, wk_g, wv_g], axis=1)).astype(np.float16)
    cosr = cos[:S].reshape(SC, 128, D)
    cos4 = np.repeat(cosr[:, :, None, :], NH, axis=2).transpose(1, 0, 2, 3).reshape(128, SC * NH * D)
    sing = np.concatenate([-sin[:S, :64], sin[:S, 64:]], axis=1).reshape(SC, 128, D)
    sin4 = np.repeat(sing[:, :, None, :], NH, axis=2).transpose(1, 0, 2, 3).reshape(128, SC * NH * D)
    wo = _chunk_major(wo_g).astype(np.float16)
    return {
        "xt": np.ascontiguousarray(xt),
        "wqkv": wqkv,
        "cos4": np.ascontiguousarray(cos4).astype(np.float16),
        "sin4": np.ascontiguousarray(sin4).astype(np.float16),
        "wo": wo,
    }


_NC_CACHE = {}

# best-measured configuration (updated as experiments conclude)
BEST_BUFS = dict(DEFAULT_BUFS)


def _get_nc(S, HID):
    key = (S, HID)
    if key not in _NC_CACHE:
        _NC_CACHE[key] = build(S, HID, bufs=BEST_BUFS)
    return _NC_CACHE[key]


def kernel(x, cos, sin, Wq, Wk, Wv, Wo):
    x = np.asarray(x, dtype=np.float32)
    cos = np.asarray(cos, dtype=np.float32)
    sin = np.asarray(sin, dtype=np.float32)
    Wq = np.asarray(Wq, dtype=np.float32)
    Wk = np.asarray(Wk, dtype=np.float32)
    Wv = np.asarray(Wv, dtype=np.float32)
    Wo = np.asarray(Wo, dtype=np.float32)
    B, S, HID = x.shape

    in_maps = []
    for i in range(8):
        b, g = i // 4, i % 4
        in_maps.append(make_in_map(
            x[b], cos, sin,
            Wq[:, g * NH * D:(g + 1) * NH * D],
            Wk[:, g * D:(g + 1) * D],
            Wv[:, g * D:(g + 1) * D],
            Wo[g * NH * D:(g + 1) * NH * D, :],
            S, HID))

    nc = _get_nc(S, HID)
    last_err = None
    for _attempt in range(3):
        try:
            res = run_bass_kernel_spmd(nc, in_maps, core_ids=list(range(8)), trace=False)
            break
        except Exception as e:  # flaky NRT_EXEC_UNIT_UNRECOVERABLE seen on first runs
            last_err = e
            import time as _time
            _time.sleep(5.0)
    else:
        raise last_err
    out = np.zeros((B, S, HID), dtype=np.float32)
    for i in range(8):
        b = i // 4
        out[b] += res.results[i]["out"]
    return out
